# revision 43
# baseline (speedup 1.0000x reference)
"""Trainium2 Bass kernel for nn_EncoderBlock (dense transformer encoder block).

Strategy (8 NeuronCores), v2 — fp8 DoubleRow matmuls:
  - Tokens sharded 512/core (cores 0-3: batch 0, cores 4-7: batch 1).
  - Activations feature-major on chip; all matmuls fp8e4m3 with
    MatmulPerfMode.DoubleRow (pairs of 128-row contraction tiles per
    instruction, 0.5 PE cycles per output column).
  - LayerNorm stats via bf16 ones-matmul partition reduction.
  - Per core: LN1 -> K^T projection, AllGather K (fp8), V projection
    (token-major), AllGather V, Q^T projection (repacked to [32,2,*] halves
    via a DRAM staging bounce for the DoubleRow score layout).
    Attention: scores [k,q] via DR (dk split 32+32), exp on Act engine with
    scale 1/8 and bias -2 (cancels in softmax, keeps e in fp8 range),
    AV via DR over k-tile pairs with a ones column for the softmax sum.
    W_o + residual, LN2, FFN (DR pairs), residual.
  - PSUM evacuations for K/V/Q on the Act engine (Copy), FFN1 on Act (Relu
    with per-partition bias); DVE handles LN + normalize + residual.
"""

import os

import numpy as np
import ml_dtypes

import concourse.bass as bass
import concourse.mybir as mybir
import concourse.tile as tile
from concourse import bacc
from concourse.bass_utils import run_bass_kernel_spmd

N_CORES = 8
GRP = 4          # cores per batch group
P = 128
TOK = 512        # tokens per core
S = 2048         # sequence length (tokens per batch)
D = 1024
KT = D // P      # 8 feature tiles
KT2 = KT // 2    # 4 pair tiles
H = 16
DK = 64
DK1 = DK + 1
VW = 80          # padded V slot width (DoubleRow needs 16B-aligned steps)
F = 4096
FT = F // P      # 32 ffn tiles
FT2 = FT // 2    # 16 ffn pair tiles
EPS = 1e-6
SCALE = 0.125    # 1/sqrt(DK)
EBIAS = -4.0     # exp bias; cancels in softmax normalization, keeps e < fp8 max
MT_S = S // P    # 16 k-token tiles per batch
NBLK = 3         # score chunks per psum tile / exp call

f32 = mybir.dt.float32
bf16 = mybir.dt.bfloat16
f8 = mybir.dt.float8e4
ALU = mybir.AluOpType
ACT = mybir.ActivationFunctionType
DR = mybir.MatmulPerfMode.DoubleRow

NP_F8 = ml_dtypes.float8_e4m3
NP_BF16 = ml_dtypes.bfloat16


def _ln_stats(nc, tc, rows, ones_b, xb_tiles, tag):
    """Per-token LN stats from bf16 feature-major tiles.

    Returns (r_bc, mr_bc): [P, TOK] f32 broadcasts of 1/std and mean/std.
    """
    ltr = tc.alloc_tile_pool(name=f"lntr_{tag}", bufs=2)
    lnps = tc.alloc_tile_pool(name=f"lnps_{tag}", bufs=1, space="PSUM")
    ps_sum = lnps.tile([1, TOK], f32, name=f"pssum_{tag}", tag="pssum")
    ps_sq = lnps.tile([1, TOK], f32, name=f"pssq_{tag}", tag="pssq")
    for kt in range(KT):
        sq = ltr.tile([P, TOK], bf16, name=f"sq_{tag}_{kt}", tag="sq", bufs=2)
        nc.vector.tensor_mul(sq[:], xb_tiles[kt][:], xb_tiles[kt][:])
        nc.tensor.matmul(ps_sum[:], lhsT=ones_b[:], rhs=xb_tiles[kt][:],
                         start=(kt == 0), stop=(kt == KT - 1))
        nc.tensor.matmul(ps_sq[:], lhsT=ones_b[:], rhs=sq[:],
                         start=(kt == 0), stop=(kt == KT - 1))

    inv_n = 1.0 / D
    mean = rows.tile([1, TOK], f32, name=f"mean_{tag}", tag="mean")
    var = rows.tile([1, TOK], f32, name=f"var_{tag}", tag="var")
    msq = rows.tile([1, TOK], f32, name=f"msq_{tag}", tag="msqlnv")
    nc.vector.tensor_scalar_mul(mean[:], ps_sum[:], inv_n)
    nc.vector.tensor_scalar_mul(var[:], ps_sq[:], inv_n)
    nc.vector.tensor_mul(msq[:], mean[:], mean[:])
    nc.vector.tensor_sub(var[:], var[:], msq[:])
    nc.vector.tensor_scalar_add(var[:], var[:], EPS)
    # r = 1/sqrt(var+eps) = exp(-0.5*ln(var+eps))
    lnv = rows.tile([1, TOK], f32, name=f"lnv_{tag}", tag="msqlnv")
    nc.scalar.activation(lnv[:], var[:], ACT.Ln)
    r_row = rows.tile([1, TOK], f32, name=f"r_{tag}", tag="r")
    nc.scalar.activation(r_row[:], lnv[:], ACT.Exp, scale=-0.5)
    mr_row = rows.tile([1, TOK], f32, name=f"mr_{tag}", tag="mr")
    nc.vector.tensor_mul(mr_row[:], mean[:], r_row[:])

    r_bc = rows.tile([P, TOK], f32, name=f"rbc_{tag}", tag=f"rbc_{tag}")
    mr_bc = rows.tile([P, TOK], f32, name=f"mrbc_{tag}", tag=f"mrbc_{tag}")
    nc.gpsimd.partition_broadcast(r_bc[:], r_row[:])
    nc.gpsimd.partition_broadcast(mr_bc[:], mr_row[:])
    lnps.release()
    ltr.release()
    return r_bc, mr_bc


def _ln_apply(nc, tc, hp, x_tiles, r_bc, mr_bc, g_t, b_t, tag, dtype=f8,
              pair=True):
    """LN output from f32 x tiles: (x*r - mr)*g + b.

    pair=True: KT2 pair tiles [P, 2*TOK] (fp8 DoubleRow layout).
    pair=False: KT separate [P, TOK] tiles (bf16 plain matmul rhs).
    """
    ltr = tc.alloc_tile_pool(name=f"lnap_{tag}", bufs=2)
    if pair:
        tiles = [hp.tile([P, 2 * TOK], dtype, name=f"h_{tag}_{k}", tag=f"h_{k}")
                 for k in range(KT2)]
        views = [tiles[kt // 2][:, (kt % 2) * TOK:(kt % 2 + 1) * TOK]
                 for kt in range(KT)]
    else:
        tiles = [hp.tile([P, TOK], dtype, name=f"h_{tag}_{k}", tag=f"h_{k}")
                 for k in range(KT)]
        views = [t[:] for t in tiles]
    for kt in range(KT):
        t1 = ltr.tile([P, TOK], f32, name=f"t1_{tag}_{kt}", tag="lnt1", bufs=2)
        nc.vector.tensor_mul(t1[:], x_tiles[kt][:], r_bc[:])
        nc.vector.tensor_sub(t1[:], t1[:], mr_bc[:])
        nc.vector.tensor_scalar(views[kt], t1[:], g_t[:, kt:kt + 1],
                                b_t[:, kt:kt + 1], ALU.mult, ALU.add)
    ltr.release()
    return tiles


SKIP_ATT = bool(int(os.environ.get("SKIP_ATT", "0")))
SKIP_FFN = bool(int(os.environ.get("SKIP_FFN", "0")))
SKIP_QKV = bool(int(os.environ.get("SKIP_QKV", "0")))


def build(n_iters: int = 1):
    nc = bacc.Bacc("TRN2", target_bir_lowering=False, debug=False,
                   num_devices=N_CORES)

    xT = nc.dram_tensor("xT", [D, TOK], f32, kind="ExternalInput").ap()
    xTb = nc.dram_tensor("xTb", [D, TOK], bf16, kind="ExternalInput").ap()
    wqd = nc.dram_tensor("wqd", [P, KT2 * 2 * D], f8, kind="ExternalInput").ap()
    wkd = nc.dram_tensor("wkd", [P, KT2 * 2 * D], f8, kind="ExternalInput").ap()
    wvd = nc.dram_tensor("wvd", [P, KT2 * 2 * D], f8, kind="ExternalInput").ap()
    wod = nc.dram_tensor("wod", [P, KT2 * 2 * D], f8, kind="ExternalInput").ap()
    w1b = nc.dram_tensor("w1b", [D, F], bf16, kind="ExternalInput").ap()
    w2b = nc.dram_tensor("w2b", [F, D], bf16, kind="ExternalInput").ap()
    bo_v = nc.dram_tensor("bo_v", [P, KT], f32, kind="ExternalInput").ap()
    b1_v = nc.dram_tensor("b1_v", [P, FT], f32, kind="ExternalInput").ap()
    b2_v = nc.dram_tensor("b2_v", [P, KT], f32, kind="ExternalInput").ap()
    g1_v = nc.dram_tensor("g1_v", [P, KT], f32, kind="ExternalInput").ap()
    be1_v = nc.dram_tensor("be1_v", [P, KT], f32, kind="ExternalInput").ap()
    g2_v = nc.dram_tensor("g2_v", [P, KT], f32, kind="ExternalInput").ap()
    be2_v = nc.dram_tensor("be2_v", [P, KT], f32, kind="ExternalInput").ap()

    outT = nc.dram_tensor("outT", [D, TOK], f32, kind="ExternalOutput").ap()

    groups = [[0, 1, 2, 3], [4, 5, 6, 7]]

    with tile.TileContext(nc) as tc:
        sb = tc.alloc_tile_pool(name="sb", bufs=1)
        tr = tc.alloc_tile_pool(name="tr", bufs=3)
        dram = tc.alloc_tile_pool(name="dram", bufs=1, space="DRAM")
        xp = tc.alloc_tile_pool(name="xp", bufs=1)

        ones_b = sb.tile([P, 1], bf16, name="ones_b", tag="ones_b")
        nc.vector.memset(ones_b[:], 1.0)
        ebias_t = sb.tile([P, 1], f32, name="ebias_t", tag="ebias_t")
        nc.vector.memset(ebias_t[:], EBIAS)
        bo_t = sb.tile([P, KT], f32, name="bo_t", tag="bo_t")
        b1_t = sb.tile([P, FT], f32, name="b1_t", tag="b1_t")
        b2_t = sb.tile([P, KT], f32, name="b2_t", tag="b2_t")
        g1_t = sb.tile([P, KT], f32, name="g1_t", tag="g1_t")
        be1_t = sb.tile([P, KT], f32, name="be1_t", tag="be1_t")
        g2_t = sb.tile([P, KT], f32, name="g2_t", tag="g2_t")
        be2_t = sb.tile([P, KT], f32, name="be2_t", tag="be2_t")
        for t, src in [(bo_t, bo_v), (b1_t, b1_v), (b2_t, b2_v),
                       (g1_t, g1_v), (be1_t, be1_v), (g2_t, g2_v),
                       (be2_t, be2_v)]:
            nc.scalar.dma_start(out=t[:], in_=src)

        x_tiles = []
        for kt in range(KT):
            xt = xp.tile([P, TOK], f32, name=f"x_{kt}", tag=f"x_{kt}")
            nc.scalar.dma_start(out=xt[:], in_=xT[kt * P:(kt + 1) * P, :])
            x_tiles.append(xt)

        kT_sh = dram.tile([D, TOK], f8, name="kT_sh", tag="kT_sh")
        kT_full = dram.tile([GRP * D, TOK], f8, name="kT_full", tag="kT_full")
        v_sh = dram.tile([TOK, D], f8, name="v_sh", tag="v_sh")
        v_full = dram.tile([S, D], f8, name="v_full", tag="v_full")
        q_stage = dram.tile([D, TOK], f8, name="q_stage", tag="q_stage")

        for it in range(n_iters):
            # pool stack (LIFO release discipline), longest-lived first
            rows = tc.alloc_tile_pool(name=f"rows{it}", bufs=1)
            x2p = tc.alloc_tile_pool(name=f"x2p{it}", bufs=1)
            wf = tc.alloc_tile_pool(name=f"wf{it}", bufs=12)
            qp = tc.alloc_tile_pool(name=f"qp{it}", bufs=1)
            ctxp = tc.alloc_tile_pool(name=f"ctxp{it}", bufs=1)
            wop = tc.alloc_tile_pool(name=f"wop{it}", bufs=1)
            xbp = tc.alloc_tile_pool(name=f"xbp{it}", bufs=1)
            hp = tc.alloc_tile_pool(name=f"hp{it}", bufs=1)
            wqkv = tc.alloc_tile_pool(name=f"wqkv{it}", bufs=1)

            def w1_tile(mg, kt):
                t = wf.tile([P, D], bf16, name=f"w1_{it}_{mg}_{kt}", tag="w")
                nc.gpsimd.dma_start(out=t[:],
                                  in_=w1b[kt * P:(kt + 1) * P,
                                          mg * D:(mg + 1) * D])
                return t

            def w2_tile(mg, kt):
                t = wf.tile([P, D], bf16, name=f"w2_{it}_{mg}_{kt}", tag="w")
                nc.gpsimd.dma_start(out=t[:],
                                  in_=w2b[(mg * KT + kt) * P:
                                          (mg * KT + kt + 1) * P, :])
                return t

            xb_tiles = []
            for kt in range(KT):
                xbt = xbp.tile([P, TOK], bf16, name=f"xb_{it}_{kt}", tag=f"xb_{kt}")
                nc.scalar.dma_start(out=xbt[:], in_=xTb[kt * P:(kt + 1) * P, :])
                xb_tiles.append(xbt)

            # weights for K first (it feeds the collective), then V, Q
            wk_t, wv_t, wq_t, wo_t = [], [], [], []
            for kt2 in range(KT2):
                t = wqkv.tile([P, 2 * D], f8, name=f"wk_{it}_{kt2}", tag=f"wk_{kt2}")
                nc.scalar.dma_start(out=t[:], in_=wkd[:, kt2 * 2 * D:(kt2 + 1) * 2 * D])
                wk_t.append(t)
            for kt2 in range(KT2):
                t = wqkv.tile([P, 2 * D], f8, name=f"wv_{it}_{kt2}", tag=f"wv_{kt2}")
                nc.scalar.dma_start(out=t[:], in_=wvd[:, kt2 * 2 * D:(kt2 + 1) * 2 * D])
                wv_t.append(t)
            for kt2 in range(KT2):
                t = wqkv.tile([P, 2 * D], f8, name=f"wq_{it}_{kt2}", tag=f"wq_{kt2}")
                nc.scalar.dma_start(out=t[:], in_=wqd[:, kt2 * 2 * D:(kt2 + 1) * 2 * D])
                wq_t.append(t)

            # ---- LN1 ----
            r1_bc, mr1_bc = _ln_stats(nc, tc, rows, ones_b, xb_tiles, f"ln1_{it}")
            h_pairs = _ln_apply(nc, tc, hp, x_tiles, r1_bc, mr1_bc,
                                g1_t, be1_t, f"ln1_{it}")

            # ---- K^T projection -> AllGather ----
            qkvps = tc.alloc_tile_pool(name=f"qkvps{it}", bufs=6, space="PSUM")
            for mt in range(KT):
                ps = qkvps.tile([P, TOK], f32, name=f"psk_{it}_{mt}", tag="qkv")
                for kt2 in range(KT2):
                    nc.tensor.matmul(
                        ps[:],
                        lhsT=wk_t[kt2][:].rearrange("p (j m) -> p j m", j=2)
                        [:, :, mt * P:(mt + 1) * P],
                        rhs=h_pairs[kt2][:].rearrange("p (j t) -> p j t", j=2),
                        start=(kt2 == 0), stop=(kt2 == KT2 - 1), perf_mode=DR)
                kev = tr.tile([P, TOK], f8, name=f"kev_{it}_{mt}", tag="kev", bufs=2)
                nc.scalar.activation(kev[:], ps[:], ACT.Copy)
                nc.gpsimd.dma_start(out=kT_sh[mt * P:(mt + 1) * P, :], in_=kev[:])
            nc.gpsimd.collective_compute(
                "AllGather", ALU.bypass, ins=[kT_sh[:].opt()],
                outs=[kT_full[:].opt()], replica_groups=groups)

            # ---- V projection (token-major) -> AllGather ----
            for mtk in range(TOK // P):
                for nt in range(2):
                    ps = qkvps.tile([P, TOK], f32, name=f"psv_{it}_{mtk}_{nt}",
                                    tag="qkv")
                    for kt2 in range(KT2):
                        nc.tensor.matmul(
                            ps[:],
                            lhsT=h_pairs[kt2][:].rearrange("p (j t) -> p j t", j=2)
                            [:, :, mtk * P:(mtk + 1) * P],
                            rhs=wv_t[kt2][:].rearrange("p (j n) -> p j n", j=2)
                            [:, :, nt * TOK:(nt + 1) * TOK],
                            start=(kt2 == 0), stop=(kt2 == KT2 - 1), perf_mode=DR)
                    vev = tr.tile([P, TOK], f8, name=f"vev_{it}_{mtk}_{nt}",
                                  tag="vev", bufs=2)
                    nc.scalar.activation(vev[:], ps[:], ACT.Copy)
                    nc.gpsimd.dma_start(
                        out=v_sh[mtk * P:(mtk + 1) * P, nt * TOK:(nt + 1) * TOK],
                        in_=vev[:])
            nc.gpsimd.collective_compute(
                "AllGather", ALU.bypass, ins=[v_sh[:].opt()],
                outs=[v_full[:].opt()], replica_groups=groups)

            # ---- Q^T projection -> repack to [32, 2, *] halves via DRAM ----
            q_pairs = []
            for p in range(KT):
                qt = qp.tile([32, 4 * TOK], f8, name=f"qpair_{it}_{p}",
                             tag=f"qpair_{p}")
                q_pairs.append(qt)
            for mt in range(KT):
                ps = qkvps.tile([P, TOK], f32, name=f"psq_{it}_{mt}", tag="qkv")
                for kt2 in range(KT2):
                    nc.tensor.matmul(
                        ps[:],
                        lhsT=wq_t[kt2][:].rearrange("p (j m) -> p j m", j=2)
                        [:, :, mt * P:(mt + 1) * P],
                        rhs=h_pairs[kt2][:].rearrange("p (j t) -> p j t", j=2),
                        start=(kt2 == 0), stop=(kt2 == KT2 - 1), perf_mode=DR)
                qev = tr.tile([P, TOK], f8, name=f"qev_{it}_{mt}", tag="qev", bufs=2)
                nc.scalar.activation(qev[:], ps[:], ACT.Copy)
                nc.gpsimd.dma_start(out=q_stage[mt * P:(mt + 1) * P, :], in_=qev[:])
                # bounce back repacked: rows (hf j i) t -> partition i,
                # free (hf, j, t); one DMA per half (3-dim AP limit)
                for hf in range(2):
                    nc.sync.dma_start(
                        out=q_pairs[mt][:].rearrange("i (hf j t) -> i hf j t",
                                                     hf=2, j=2)[:, hf],
                        in_=q_stage[mt * P:(mt + 1) * P, :]
                        .rearrange("(hf j i) t -> i hf j t", hf=2, j=2)[:, hf])
            qkvps.release()
            wqkv.release()
            hp.release()
            xbp.release()

            # ---- attention ----
            for kt2 in range(KT2):
                t = wop.tile([P, 2 * D], f8, name=f"wo_{it}_{kt2}", tag=f"wo_{kt2}")
                nc.gpsimd.dma_start(out=t[:], in_=wod[:, kt2 * 2 * D:(kt2 + 1) * 2 * D])
                wo_t.append(t)

            kvp = tc.alloc_tile_pool(name=f"kvp{it}", bufs=3)
            ep = tc.alloc_tile_pool(name=f"ep{it}", bufs=3)
            scps = tc.alloc_tile_pool(name=f"scps{it}", bufs=2, space="PSUM")
            ctxps = tc.alloc_tile_pool(name=f"ctxps{it}", bufs=2, space="PSUM")

            ctx_pairs = []
            for t2 in range(KT2):
                ct = ctxp.tile([P, 2 * TOK], f8, name=f"ctx_{it}_{t2}",
                               tag=f"ctx_{t2}")
                ctx_pairs.append(ct)

            kT_view = kT_full[:].rearrange("(c pp hf j i) t -> pp hf i j c t",
                                           c=GRP, pp=KT, hf=2, j=2, i=32)

            w1g0, w2g0 = [], []
            for p in range(KT):
                # prefetch first FFN weight group spread across attention
                for idx in (2 * p, 2 * p + 1):
                    if idx < KT:
                        w1g0.append(w1_tile(0, idx))
                    else:
                        w2g0.append(w2_tile(0, idx - KT))

                # per-slot width padded 65 -> 80 for DoubleRow 16B step align
                if SKIP_ATT:
                    nc.vector.memset(ctx_pairs[p // 2][:, (p % 2) * TOK:
                                                       (p % 2 + 1) * TOK], 0.01)
                    continue
                vp = kvp.tile([P, MT_S * VW * 2], f8, name=f"vp_{it}_{p}",
                              tag="vp", bufs=3)
                vp_v = vp[:].rearrange("q (m j hf c) -> q m j hf c",
                                       m=MT_S // 2, j=2, hf=2)
                for hf in range(2):
                    for jv in range(2):
                        nc.sync.dma_start(
                            out=vp_v[:, :, jv, hf, 0:DK],
                            in_=v_full[:, p * P + hf * DK:p * P + (hf + 1) * DK]
                            .rearrange("(m j q) c -> q m j c",
                                       m=MT_S // 2, j=2)[:, :, jv])
                nc.vector.memset(vp_v[:, :, :, :, DK:DK1].squeeze(4), 1.0)

                q_v = q_pairs[p][:].rearrange("i (hf j t) -> i hf j t", hf=2, j=2)

                for hf in range(2):
                    # layout [i, (c, j, t)]: chunk m lives at c=m//4, t-block
                    # m%4; j stride 512 satisfies the DoubleRow 16B alignment
                    ktp = kvp.tile([32, 2 * S], f8, name=f"ktp_{it}_{p}_{hf}",
                                   tag="ktp", bufs=3)
                    for jk in range(2):
                        nc.sync.dma_start(
                            out=ktp[:].rearrange("i (c j t) -> i c j t",
                                                 c=GRP, j=2)[:, :, jk],
                            in_=kT_view[p, hf, :, jk])
                    ktp_v = ktp[:].rearrange("i (c j t) -> i c j t", c=GRP, j=2)

                    e_all = ep.tile([P, MT_S * TOK], f8, name=f"e_{it}_{p}_{hf}",
                                    tag="e")
                    ps_ctx = ctxps.tile([DK1, TOK], f32,
                                        name=f"psctx_{it}_{p}_{hf}", tag="psctx")
                    # software-pipelined: scores block b, exp block b, then AV
                    # pairs from block b-1 (so stalled AVs never delay the
                    # next score block ahead of the Act engine's exp stream)
                    next_pair = 0
                    nblocks = (MT_S + NBLK - 1) // NBLK
                    for b in range(nblocks):
                        i0 = b * NBLK
                        nb = min(NBLK, MT_S - i0)
                        ps_sc = scps.tile([P, NBLK * TOK], f32,
                                          name=f"pssc_{it}_{p}_{hf}_{b}",
                                          tag="pssc")
                        for i in range(i0, i0 + nb):
                            nc.tensor.matmul(
                                ps_sc[:, (i - i0) * TOK:(i - i0 + 1) * TOK],
                                lhsT=ktp_v[:, i // 4, :,
                                           (i % 4) * P:(i % 4 + 1) * P],
                                rhs=q_v[:, hf],
                                start=True, stop=True, perf_mode=DR)
                        nc.scalar.activation(
                            e_all[:, i0 * TOK:(i0 + nb) * TOK],
                            ps_sc[:, 0:nb * TOK],
                            ACT.Exp, scale=SCALE, bias=ebias_t[:])
                        # AV pairs fully covered by chunks < i0 (prev blocks)
                        while (next_pair + 1) * 2 <= i0:
                            m2 = next_pair
                            nc.tensor.matmul(
                                ps_ctx[:],
                                lhsT=vp_v[:, m2, :, hf, 0:DK1],
                                rhs=e_all[:, m2 * 2 * TOK:(m2 + 1) * 2 * TOK]
                                .rearrange("k (j t) -> k j t", j=2),
                                start=(m2 == 0), stop=(m2 == MT_S // 2 - 1),
                                perf_mode=DR)
                            next_pair += 1
                    while (next_pair + 1) * 2 <= MT_S:
                        m2 = next_pair
                        nc.tensor.matmul(
                            ps_ctx[:],
                            lhsT=vp_v[:, m2, :, hf, 0:DK1],
                            rhs=e_all[:, m2 * 2 * TOK:(m2 + 1) * 2 * TOK]
                            .rearrange("k (j t) -> k j t", j=2),
                            start=(m2 == 0), stop=(m2 == MT_S // 2 - 1),
                            perf_mode=DR)
                        next_pair += 1
                    # normalize: ctx = ctx_unnorm / sumexp
                    t2, jj = p // 2, p % 2
                    rec = tr.tile([1, TOK], f32, name=f"rec_{it}_{p}_{hf}",
                                  tag="rec", bufs=2)
                    nc.vector.reciprocal(rec[:], ps_ctx[DK:DK1, :])
                    rbc = tr.tile([DK, TOK], f32, name=f"rbc_{it}_{p}_{hf}",
                                  tag="recbc", bufs=2)
                    nc.gpsimd.partition_broadcast(rbc[:], rec[:])
                    if hf == 0:
                        nc.vector.tensor_mul(
                            ctx_pairs[t2][0:DK, jj * TOK:(jj + 1) * TOK],
                            ps_ctx[0:DK, :], rbc[:])
                    else:
                        shift = tr.tile([DK, TOK], f8, name=f"sh_{it}_{p}",
                                        tag="shift", bufs=2)
                        nc.vector.tensor_mul(shift[:], ps_ctx[0:DK, :], rbc[:])
                        nc.sync.dma_start(
                            out=ctx_pairs[t2][DK:P, jj * TOK:(jj + 1) * TOK],
                            in_=shift[:])
            ctxps.release()
            scps.release()
            ep.release()
            kvp.release()

            # ---- W_o + residual -> x2 ----
            wops = tc.alloc_tile_pool(name=f"wops{it}", bufs=6, space="PSUM")
            x2_tiles = []
            for mt in range(KT):
                ps = wops.tile([P, TOK], f32, name=f"pso_{it}_{mt}", tag="wo")
                for t2 in range(KT2):
                    nc.tensor.matmul(
                        ps[:],
                        lhsT=wo_t[t2][:].rearrange("p (j m) -> p j m", j=2)
                        [:, :, mt * P:(mt + 1) * P],
                        rhs=ctx_pairs[t2][:].rearrange("p (j t) -> p j t", j=2),
                        start=(t2 == 0), stop=(t2 == KT2 - 1), perf_mode=DR)
                x2 = x2p.tile([P, TOK], f32, name=f"x2_{it}_{mt}", tag=f"x2_{mt}")
                nc.vector.scalar_tensor_tensor(x2[:], ps[:], bo_t[:, mt:mt + 1],
                                               x_tiles[mt][:], ALU.add, ALU.add)
                x2_tiles.append(x2)
            wops.release()
            wop.release()
            ctxp.release()
            qp.release()

            # ---- LN2 ----
            x2bp = tc.alloc_tile_pool(name=f"x2bp{it}", bufs=1)
            x2b_tiles = []
            for kt in range(KT):
                xbt = x2bp.tile([P, TOK], bf16, name=f"x2b_{it}_{kt}",
                                tag=f"x2b_{kt}")
                nc.scalar.activation(xbt[:], x2_tiles[kt][:], ACT.Copy)
                x2b_tiles.append(xbt)
            r2_bc, mr2_bc = _ln_stats(nc, tc, rows, ones_b, x2b_tiles, f"ln2_{it}")
            x2bp.release()
            h2p = tc.alloc_tile_pool(name=f"h2p{it}", bufs=1)
            h2_tiles = _ln_apply(nc, tc, h2p, x2_tiles, r2_bc, mr2_bc,
                                 g2_t, be2_t, f"ln2_{it}", dtype=bf16, pair=False)

            # ---- FFN (bf16 for accuracy; fp8 here breaks the error budget) ----
            apl = tc.alloc_tile_pool(name=f"apl{it}", bufs=16)
            f1ps = tc.alloc_tile_pool(name=f"f1ps{it}", bufs=4, space="PSUM")
            f2ps = tc.alloc_tile_pool(name=f"f2ps{it}", bufs=4, space="PSUM")
            for mg in range(0 if SKIP_FFN else 4):
                w1_tiles = w1g0 if mg == 0 else [w1_tile(mg, kt)
                                                 for kt in range(KT)]
                w2_tiles = w2g0 if mg == 0 else [w2_tile(mg, kt)
                                                 for kt in range(KT)]
                a_tiles = []
                for mt in range(KT):
                    m = mg * KT + mt
                    ps = f1ps.tile([P, TOK], f32, name=f"psf1_{it}_{m}", tag="f1")
                    for kt in range(KT):
                        nc.tensor.matmul(
                            ps[:], lhsT=w1_tiles[kt][:, mt * P:(mt + 1) * P],
                            rhs=h2_tiles[kt][:],
                            start=(kt == 0), stop=(kt == KT - 1))
                    at = apl.tile([P, TOK], bf16, name=f"a_{it}_{m}", tag="a")
                    nc.scalar.activation(at[:], ps[:], ACT.Relu,
                                         bias=b1_t[:, m:m + 1])
                    a_tiles.append(at)
                for mt in range(KT):
                    ps2 = f2ps.tile([P, TOK], f32, name=f"psf2_{it}_{mg}_{mt}",
                                    tag="f2")
                    for kt in range(KT):
                        nc.tensor.matmul(
                            ps2[:], lhsT=w2_tiles[kt][:, mt * P:(mt + 1) * P],
                            rhs=a_tiles[kt][:],
                            start=(kt == 0), stop=(kt == KT - 1))
                    nc.vector.tensor_add(x2_tiles[mt][:], x2_tiles[mt][:],
                                         ps2[:])
            f2ps.release()
            f1ps.release()
            apl.release()
            h2p.release()

            # ---- final bias + store ----
            for mt in range(KT):
                ot = tr.tile([P, TOK], f32, name=f"ot_{it}_{mt}", tag="ot", bufs=2)
                nc.vector.tensor_scalar_add(ot[:], x2_tiles[mt][:],
                                            b2_t[:, mt:mt + 1])
                nc.gpsimd.dma_start(out=outT[mt * P:(mt + 1) * P, :], in_=ot[:])
            wf.release()
            x2p.release()
            rows.release()

        xp.release()
        dram.release()
        tr.release()
        sb.release()

    nc.compile()
    return nc


_NC_CACHE = {}


def _get_nc(n_iters: int = 1):
    if n_iters not in _NC_CACHE:
        _NC_CACHE[n_iters] = build(n_iters)
    return _NC_CACHE[n_iters]


def _pack_vec(v: np.ndarray, nt: int) -> np.ndarray:
    return np.ascontiguousarray(v.reshape(nt, P).T, dtype=np.float32)


def _pack_pairs(w: np.ndarray, npair: int, ncols: int) -> np.ndarray:
    """[K, M] -> [P, npair*2*M] fp8 with K = npair*2*P paired rows."""
    w = np.asarray(w, dtype=np.float32)
    w = w.reshape(npair, 2, P, ncols).transpose(2, 0, 1, 3)
    return np.ascontiguousarray(w.reshape(P, npair * 2 * ncols)).astype(NP_F8)


def prep_in_maps(inputs: dict) -> list:
    x = np.asarray(inputs["x"], dtype=np.float32)
    B, S_, D_ = x.shape
    assert (B, S_, D_) == (2, S, D)
    xf = x.reshape(B * S_, D_)

    shared = {
        "wqd": _pack_pairs(inputs["W_q"], KT2, D),
        "wkd": _pack_pairs(inputs["W_k"], KT2, D),
        "wvd": _pack_pairs(inputs["W_v"], KT2, D),
        "wod": _pack_pairs(inputs["W_o"], KT2, D),
        "w1b": np.ascontiguousarray(np.asarray(inputs["W1"], np.float32)
                                    .astype(NP_BF16)),
        "w2b": np.ascontiguousarray(np.asarray(inputs["W2"], np.float32)
                                    .astype(NP_BF16)),
        "bo_v": _pack_vec(np.asarray(inputs["b_o"], np.float32), KT),
        "b1_v": _pack_vec(np.asarray(inputs["b1"], np.float32), FT),
        "b2_v": _pack_vec(np.asarray(inputs["b2"], np.float32), KT),
        "g1_v": _pack_vec(np.asarray(inputs["ln1_g"], np.float32), KT),
        "be1_v": _pack_vec(np.asarray(inputs["ln1_b"], np.float32), KT),
        "g2_v": _pack_vec(np.asarray(inputs["ln2_g"], np.float32), KT),
        "be2_v": _pack_vec(np.asarray(inputs["ln2_b"], np.float32), KT),
    }
    in_maps = []
    for c in range(N_CORES):
        xT_c = np.ascontiguousarray(xf[c * TOK:(c + 1) * TOK, :].T)
        in_maps.append({"xT": xT_c, "xTb": xT_c.astype(NP_BF16), **shared})
    return in_maps


def run(inputs: dict, trace: bool = False):
    in_maps = prep_in_maps(inputs)
    nc = _get_nc(1)
    res = run_bass_kernel_spmd(nc, in_maps, list(range(N_CORES)), trace=trace)
    B, S_, D_ = np.asarray(inputs["x"]).shape
    out = np.empty((B * S_, D_), dtype=np.float32)
    for c in range(N_CORES):
        out[c * TOK:(c + 1) * TOK, :] = res.results[c]["outT"].T
    return out.reshape(B, S_, D_), res


def kernel(**inputs) -> np.ndarray:
    out, _ = run(inputs, trace=False)
    return out
